# revision 7
# baseline (speedup 1.0000x reference)
"""Trainium2 Bass kernel for nn_MixquantLinear: O = ((dequant4(V) * S) @ dequant4(U)).T.

Output O is [4096, 4096] fp32, built purely from the GPTQ-quantized weights
(the activation input `x` is dead code in the reference and never touches the
device). Sharding: 4 slices over the output rows (o-dim) x 2 slices over the
output cols (i-dim) -> 8 cores, no collectives; host concatenates the blocks.

Per core:
  - unpack 4-bit nibbles (vector engine shift+mask, int32 words)
  - dequant affine (q - (z+1)) * scale with S folded into V's scale;
    chunks split between vector and scalar engines (per-partition scalars)
  - PE-transpose of the U slice into [rank, out] lhsT layout
  - fp16 matmuls (k-tiles of 128, N=512) accumulating fp32 in PSUM
  - wide scalar-engine PSUM->SBUF output copies, DMA out
Host-side work is layout-only (slicing/transposing packed int32 words and
fp32 scale tables, concatenating outputs).
"""

import numpy as np

import concourse.bass as bass
import concourse.mybir as mybir
import concourse.tile as tile
from concourse import bacc
from concourse.bass_utils import run_bass_kernel_spmd
from concourse.masks import make_identity

IN_SIZE = 4096
OUT_SIZE = 4096
RANK = 1024
GROUPSIZE = 128
PACK = 8
P_O = 4
P_I = 2
O_SL = OUT_SIZE // P_O    # 1024
I_SL = IN_SIZE // P_I     # 2048
N_CORES = P_O * P_I
KT = RANK // 128          # 8
RT = KT
OT = O_SL // 128          # 8
N_STRIPS = 2
STRIP = I_SL // N_STRIPS  # 1024
GV = I_SL // GROUPSIZE    # 16 i-groups per core
GU = RANK // GROUPSIZE    # 8 r-groups

F16 = mybir.dt.float16
F32 = mybir.dt.float32
I32 = mybir.dt.int32
Alu = mybir.AluOpType
Act = mybir.ActivationFunctionType

_NC_CACHE = None
TRACE = False
LAST_RESULTS = None

# affine work split: of every (AFF_DVE + AFF_ACT) chunks, this many go to DVE
AFF_DVE = 3
AFF_ACT = 2


def _build_nc():
    nc = bacc.Bacc("TRN2", target_bir_lowering=False)

    qvt = nc.dram_tensor("qvt", [128, RT * (I_SL // PACK)], I32, kind="ExternalInput")
    svt = nc.dram_tensor("svt", [128, RT * GV], F32, kind="ExternalInput")
    qzv = nc.dram_tensor("qzv", [GV, RANK // PACK], I32, kind="ExternalInput")
    qut = nc.dram_tensor("qut", [128, OT * (RANK // PACK)], I32, kind="ExternalInput")
    sut = nc.dram_tensor("sut", [128, OT * GU], F32, kind="ExternalInput")
    qzu = nc.dram_tensor("qzu", [GU, O_SL // PACK], I32, kind="ExternalInput")
    s_in = nc.dram_tensor("s", [128, RT], F32, kind="ExternalInput")
    out = nc.dram_tensor("out", [O_SL, I_SL], F32, kind="ExternalOutput")

    aff_n = {"n": 0}
    cp_n = {"n": 0}

    def affine(out_ap, in_ap, zeff_col, a_col, bvn_col):
        """out = (in - zeff) * a, weighted-split across DVE and ACT."""
        i = aff_n["n"]
        aff_n["n"] += 1
        if i % (AFF_DVE + AFF_ACT) < AFF_DVE:
            nc.vector.tensor_scalar(
                out=out_ap, in0=in_ap, scalar1=zeff_col, scalar2=a_col,
                op0=Alu.subtract, op1=Alu.mult,
            )
        else:
            nc.scalar.activation(out_ap, in_ap, Act.Identity, bias=bvn_col, scale=a_col)

    def copy_alt(out_ap, in_ap):
        cp_n["n"] += 1
        if cp_n["n"] % 2 == 0:
            nc.scalar.copy(out_ap, in_ap)
        else:
            nc.vector.tensor_copy(out_ap, in_ap)

    with tile.TileContext(nc) as tc:
        with (
            tc.tile_pool(name="const", bufs=1) as cp,
            tc.tile_pool(name="nibs", bufs=3) as nibp,
            tc.tile_pool(name="outsb", bufs=4) as outp,
        ):
            qvt_sb = cp.tile([128, RT * (I_SL // PACK)], I32, tag="qvt")
            qut_sb = cp.tile([128, OT * (RANK // PACK)], I32, tag="qut")
            svt_sb = cp.tile([128, RT * GV], F32, tag="svt")
            sut_sb = cp.tile([128, OT * GU], F32, tag="sut")
            s_sb = cp.tile([128, RT], F32, tag="s")
            qzv_sb = cp.tile([GV, RANK // PACK], I32, tag="qzv")
            qzu_sb = cp.tile([GU, O_SL // PACK], I32, tag="qzu")
            zv_unp = cp.tile([GV, RANK], I32, tag="zvu")
            zu_unp = cp.tile([GU, O_SL], I32, tag="zuu")
            zv_f = cp.tile([GV, RANK], F32, tag="zvf")
            zu_f = cp.tile([GU, O_SL], F32, tag="zuf")
            zeffv = cp.tile([128, RT * GV], F32, tag="zeffv")
            zeffu = cp.tile([128, OT * GU], F32, tag="zeffu")
            av = cp.tile([128, RT * GV], F32, tag="av")
            bvnv = cp.tile([128, RT * GV], F32, tag="bvnv")
            bvnu = cp.tile([128, OT * GU], F32, tag="bvnu")
            id16 = cp.tile([128, 128], F16, tag="id16")
            id32 = cp.tile([128, 128], F32, tag="id32")
            wut = cp.tile([128, OT * RANK], F16, tag="wut")
            lhsT = cp.tile([128, KT * O_SL], F16, tag="lhsT")
            rhs = [cp.tile([128, RT * STRIP], F16, tag=f"rhs{s}", name=f"rhs{s}")
                   for s in range(N_STRIPS)]

            nc.sync.dma_start(out=qzv_sb[:], in_=qzv[:])
            nc.sync.dma_start(out=qzu_sb[:], in_=qzu[:])
            nc.sync.dma_start(out=svt_sb[:], in_=svt[:])
            nc.sync.dma_start(out=sut_sb[:], in_=sut[:])
            nc.sync.dma_start(out=s_sb[:], in_=s_in[:])
            nc.sync.dma_start(out=qut_sb[:], in_=qut[:])
            nc.sync.dma_start(out=qvt_sb[:], in_=qvt[:])

            make_identity(nc, id16[:])
            make_identity(nc, id32[:])

            # ---- zeros unpack (packed along free dim) + PE transpose to [x, g] ----
            zvu_r = zv_unp[:].rearrange("p (w j) -> p w j", j=PACK)
            zuu_r = zu_unp[:].rearrange("p (w j) -> p w j", j=PACK)
            for j in range(PACK):
                nc.vector.tensor_scalar(
                    out=zvu_r[:, :, j], in0=qzv_sb[:], scalar1=4 * j, scalar2=15,
                    op0=Alu.logical_shift_right, op1=Alu.bitwise_and)
                nc.vector.tensor_scalar(
                    out=zuu_r[:, :, j], in0=qzu_sb[:], scalar1=4 * j, scalar2=15,
                    op0=Alu.logical_shift_right, op1=Alu.bitwise_and)
            nc.vector.tensor_copy(zv_f[:], zv_unp[:])
            nc.vector.tensor_copy(zu_f[:], zu_unp[:])

            with tc.tile_pool(name="zps", bufs=2, space="PSUM") as zps:
                for t in range(RT):
                    pt = zps.tile([128, GV], F32, tag="zp")
                    nc.tensor.transpose(pt[:], zv_f[:, t * 128:(t + 1) * 128], id32[:GV, :GV])
                    nc.scalar.copy(zeffv[:, t * GV:(t + 1) * GV], pt[:])
                for t in range(OT):
                    pt = zps.tile([128, GU], F32, tag="zp")
                    nc.tensor.transpose(pt[:], zu_f[:, t * 128:(t + 1) * 128], id32[:GU, :GU])
                    nc.scalar.copy(zeffu[:, t * GU:(t + 1) * GU], pt[:])

                nc.vector.tensor_scalar(out=zeffv[:], in0=zeffv[:], scalar1=1.0,
                                        scalar2=None, op0=Alu.add)
                nc.vector.tensor_scalar(out=zeffu[:], in0=zeffu[:], scalar1=1.0,
                                        scalar2=None, op0=Alu.add)
                for t in range(RT):
                    nc.vector.tensor_scalar(
                        out=av[:, t * GV:(t + 1) * GV], in0=svt_sb[:, t * GV:(t + 1) * GV],
                        scalar1=s_sb[:, t:t + 1], scalar2=None, op0=Alu.mult)
                nc.vector.tensor_tensor(bvnv[:], zeffv[:], av[:], Alu.mult)
                nc.vector.tensor_scalar(out=bvnv[:], in0=bvnv[:], scalar1=-1.0,
                                        scalar2=None, op0=Alu.mult)
                nc.vector.tensor_tensor(bvnu[:], zeffu[:], sut_sb[:], Alu.mult)
                nc.vector.tensor_scalar(out=bvnu[:], in0=bvnu[:], scalar1=-1.0,
                                        scalar2=None, op0=Alu.mult)

                # ---- U side: unpack + affine in [o, r], PE-transpose into lhsT ----
                with tc.tile_pool(name="ups", bufs=6, space="PSUM") as ups:
                    for t in range(OT):
                        nibu = nibp.tile([128, RANK], I32, tag="nibu")
                        nibu_r = nibu[:].rearrange("p (w j) -> p w j", j=PACK)
                        words = qut_sb[:, t * (RANK // PACK):(t + 1) * (RANK // PACK)]
                        for j in range(PACK):
                            nc.vector.tensor_scalar(
                                out=nibu_r[:, :, j], in0=words, scalar1=4 * j, scalar2=15,
                                op0=Alu.logical_shift_right, op1=Alu.bitwise_and)
                        for g in range(GU):
                            col = t * GU + g
                            affine(
                                wut[:, t * RANK + g * 128: t * RANK + (g + 1) * 128],
                                nibu[:, g * 128:(g + 1) * 128],
                                zeffu[:, col:col + 1], sut_sb[:, col:col + 1],
                                bvnu[:, col:col + 1])
                        for k in range(KT):
                            pt = ups.tile([128, 128], F16, tag="up")
                            nc.tensor.transpose(
                                pt[:], wut[:, t * RANK + k * 128: t * RANK + (k + 1) * 128],
                                id16[:])
                            copy_alt(lhsT[:, k * O_SL + t * 128: k * O_SL + (t + 1) * 128],
                                     pt[:])

            # ---- V dequant + matmuls, interleaved so no engine FIFO blocks ----
            def deq_rt(st, rt):
                rs = rhs[st]
                wps = STRIP // PACK
                nibv = nibp.tile([128, STRIP], I32, tag="nibv", name="nibv")
                nibv_r = nibv[:].rearrange("p (w j) -> p w j", j=PACK)
                words = qvt_sb[:, rt * (I_SL // PACK) + st * wps:
                               rt * (I_SL // PACK) + (st + 1) * wps]
                for j in range(PACK):
                    nc.vector.tensor_scalar(
                        out=nibv_r[:, :, j], in0=words, scalar1=4 * j, scalar2=15,
                        op0=Alu.logical_shift_right, op1=Alu.bitwise_and)
                for gs in range(STRIP // GROUPSIZE):
                    col = rt * GV + st * (STRIP // GROUPSIZE) + gs
                    affine(
                        rs[:, rt * STRIP + gs * 128: rt * STRIP + (gs + 1) * 128],
                        nibv[:, gs * 128:(gs + 1) * 128],
                        zeffv[:, col:col + 1], av[:, col:col + 1],
                        bvnv[:, col:col + 1])

            def mm_group(mps, st, h, m):
                pt = mps.tile([128, 512], F32, tag="mm", name="mmps")
                rs = rhs[st]
                for k in range(KT):
                    nc.tensor.matmul(
                        pt[:],
                        lhsT[:, k * O_SL + m * 128: k * O_SL + (m + 1) * 128],
                        rs[:, k * STRIP + h * 512: k * STRIP + (h + 1) * 512],
                        start=(k == 0), stop=(k == KT - 1))
                ot = outp.tile([128, 512], F32, tag="ot", name="ot")
                copy_alt(ot[:], pt[:])
                nc.sync.dma_start(
                    out=out[m * 128:(m + 1) * 128,
                            st * STRIP + h * 512: st * STRIP + (h + 1) * 512],
                    in_=ot[:])

            with tc.tile_pool(name="mps", bufs=8, space="PSUM") as mps:
                for rt in range(RT):
                    deq_rt(0, rt)
                # strip-1 dequant interleaved with strip-0 h0 matmul wave
                for x in range(RT):
                    deq_rt(1, x)
                    mm_group(mps, 0, 0, x)
                for m in range(OT):
                    mm_group(mps, 0, 1, m)
                for h in range(STRIP // 512):
                    for m in range(OT):
                        mm_group(mps, 1, h, m)

    nc.compile()
    return nc


def _host_prep(qweight_V, qzeros_V, scales_V, qweight_U, qzeros_U, scales_U, S):
    """Layout-only host prep: slice/transpose packed words + tables into SBUF layouts."""
    in_maps = []
    for c in range(N_CORES):
        a, b = divmod(c, P_I)
        qv = qweight_V[b * (I_SL // PACK):(b + 1) * (I_SL // PACK), :]
        qvt_h = np.ascontiguousarray(
            qv.T.reshape(RT, 128, I_SL // PACK).transpose(1, 0, 2).reshape(128, -1))
        sv = scales_V.T[:, b * GV:(b + 1) * GV]
        svt_h = np.ascontiguousarray(
            sv.reshape(RT, 128, GV).transpose(1, 0, 2).reshape(128, -1))
        qzv_h = np.ascontiguousarray(qzeros_V[b * GV:(b + 1) * GV, :])
        qu = qweight_U[:, a * O_SL:(a + 1) * O_SL]
        qut_h = np.ascontiguousarray(
            qu.T.reshape(OT, 128, RANK // PACK).transpose(1, 0, 2).reshape(128, -1))
        su = scales_U.T[a * O_SL:(a + 1) * O_SL, :]
        sut_h = np.ascontiguousarray(
            su.reshape(OT, 128, GU).transpose(1, 0, 2).reshape(128, -1))
        qzu_h = np.ascontiguousarray(qzeros_U[:, a * (O_SL // PACK):(a + 1) * (O_SL // PACK)])
        s_h = np.ascontiguousarray(S.reshape(RT, 128).T)
        in_maps.append({
            "qvt": qvt_h, "svt": svt_h, "qzv": qzv_h,
            "qut": qut_h, "sut": sut_h, "qzu": qzu_h, "s": s_h,
        })
    return in_maps


def kernel(x, qweight_V, qzeros_V, scales_V, g_idx_V,
           qweight_U, qzeros_U, scales_U, g_idx_U, S, **_unused):
    global _NC_CACHE, LAST_RESULTS
    qweight_V = np.asarray(qweight_V, dtype=np.int32)
    qzeros_V = np.asarray(qzeros_V, dtype=np.int32)
    scales_V = np.asarray(scales_V, dtype=np.float32)
    qweight_U = np.asarray(qweight_U, dtype=np.int32)
    qzeros_U = np.asarray(qzeros_U, dtype=np.int32)
    scales_U = np.asarray(scales_U, dtype=np.float32)
    S = np.asarray(S, dtype=np.float32)

    if _NC_CACHE is None:
        _NC_CACHE = _build_nc()
    nc = _NC_CACHE

    in_maps = _host_prep(qweight_V, qzeros_V, scales_V,
                         qweight_U, qzeros_U, scales_U, S)
    res = run_bass_kernel_spmd(nc, in_maps, core_ids=list(range(N_CORES)), trace=TRACE)
    LAST_RESULTS = res

    O = np.empty((OUT_SIZE, IN_SIZE), dtype=np.float32)
    for c in range(N_CORES):
        a, b = divmod(c, P_I)
        O[a * O_SL:(a + 1) * O_SL, b * I_SL:(b + 1) * I_SL] = res.results[c]["out"]
    return O
